# revision 1
# baseline (speedup 1.0000x reference)
"""MAE ViT-Base encoder (masked, KEEP=49) on 8 TRN2 NeuronCores.

Data-parallel over batch (8 images/core). Feature-major activations
[768, 392] on-chip; bf16 matmul inputs, fp32 PSUM accumulation, fp32
residual stream. Only the 49 kept patches are embedded (the mask gather
happens on host before the conv matmul). LayerNorm statistics and
per-token broadcasts are computed with ones/rank-1 matmuls on the
TensorEngine; softmax is computed in transposed layout so attention
needs no on-chip transposes.
"""

import numpy as np
import ml_dtypes

import concourse.bass as bass
import concourse.mybir as mybir
import concourse.tile as tile
from concourse import bacc
from concourse.bass import ts
from concourse.bass_utils import run_bass_kernel_spmd

AFT = mybir.ActivationFunctionType
BF16 = mybir.dt.bfloat16
F32 = mybir.dt.float32
ALU = mybir.AluOpType

B, P, DIM, DEPTH, NH, DH, FF = 64, 16, 768, 12, 12, 64, 3072
KEEP = 49
NCORES = 8
BL = B // NCORES          # 8 images per core
T = BL * KEEP             # 392 tokens per core
C = DIM // 128            # 6 feature chunks
FC = FF // 128            # 24 ffn chunks
EPS = 1e-9

bf16 = ml_dtypes.bfloat16
_cached = {}


def _chunk_pack(w, cols):
    """[768, cols] -> [128, 6*cols] with tile[p, c*cols+x] = w[c*128+p, x]."""
    return np.ascontiguousarray(
        w.reshape(C, 128, cols).transpose(1, 0, 2).reshape(128, C * cols))


def _build():
    nc = bacc.Bacc("TRN2", target_bir_lowering=False, debug=False,
                   enable_asserts=False, num_devices=NCORES)

    def din(name, shape, dt=BF16):
        return nc.dram_tensor(name, shape, dt, kind="ExternalInput").ap()

    pxT = din("pxT", [128, C * T])
    convw = din("convw", [128, C * DIM])
    peL = din("peL", [KEEP, DIM])
    repI = din("repI", [KEEP, T])
    qw_d = din("qw", [DEPTH, 128, C * DIM])
    kw_d = din("kw", [DEPTH, 128, C * DIM])
    vw_d = din("vw", [DEPTH, 128, C * DIM])
    pw_d = din("pw", [DEPTH, 128, C * DIM])
    w1_d = din("w1q", [DEPTH, 4, 128, C * DIM])
    w2_d = din("w2q", [DEPTH, 128, 4 * C * DIM])
    b2_d = din("b2p", [DEPTH, 1, DIM])
    b1_d = din("b1c", [DEPTH, 128, FC], F32)
    ln1r_d = din("ln1r", [DEPTH, 1, DIM])   # ln1 scale rows
    ln1b_d = din("ln1b", [DEPTH, 1, DIM])   # -ln1 bias rows
    ln2r_d = din("ln2r", [DEPTH, 1, DIM])
    ln2b_d = din("ln2b", [DEPTH, 1, DIM])
    out_d = nc.dram_tensor("out", [DIM, T], F32, kind="ExternalOutput").ap()

    with tile.TileContext(nc) as tc:
        from contextlib import ExitStack
        es = ExitStack()
        cpool = es.enter_context(tc.tile_pool(name="consts", bufs=1))
        apool = es.enter_context(tc.tile_pool(name="acts", bufs=1))
        hpool = es.enter_context(tc.tile_pool(name="h", bufs=1))
        vpool = es.enter_context(tc.tile_pool(name="vt", bufs=5))
        wq_pool = es.enter_context(tc.tile_pool(name="wqkv", bufs=2))
        wp_pool = es.enter_context(tc.tile_pool(name="wp", bufs=1))
        w1_pool = es.enter_context(tc.tile_pool(name="w1", bufs=2))
        w2_pool = es.enter_context(tc.tile_pool(name="w2", bufs=1))
        lp_pool = es.enter_context(tc.tile_pool(name="lparam", bufs=2))
        tpool = es.enter_context(tc.tile_pool(name="tmp", bufs=3))
        r1pool = es.enter_context(tc.tile_pool(name="rows1", bufs=1))
        r2pool = es.enter_context(tc.tile_pool(name="rows2", bufs=2))
        epool = es.enter_context(tc.tile_pool(name="etiles", bufs=8))
        pspool = es.enter_context(tc.tile_pool(name="ps", bufs=8, space="PSUM"))

        def psum(shape=(128, T)):
            return pspool.tile(list(shape), F32, tag="ps", name="ps")

        # ---- small constants
        ones128 = cpool.tile([128, 1], BF16, tag="ones128", name="ones128")
        nc.vector.memset(ones128[:], 1.0)
        ones49 = cpool.tile([KEEP, 1], BF16, tag="ones49", name="ones49")
        nc.vector.memset(ones49[:], 1.0)
        onesr1 = cpool.tile([1, 128], BF16, tag="onesr1", name="onesr1")
        nc.vector.memset(onesr1[:], 1.0)
        onesrT = cpool.tile([1, T], BF16, tag="onesrT", name="onesrT")
        nc.vector.memset(onesrT[:], 1.0)
        eps1 = cpool.tile([1, 1], F32, tag="eps1", name="eps1")
        nc.vector.memset(eps1[:], EPS)
        zer49 = cpool.tile([KEEP, 1], F32, tag="zer49", name="zer49")
        nc.vector.memset(zer49[:], 0.0)

        # ---- patch-embed constants share the w2 slot (freed before layer 0 FFN)
        NPX, NCW, NPE, NRI = C * T, C * DIM, DIM, T
        cst = w2_pool.tile([128, NPX + NCW + NPE + NRI], BF16, tag="w2", name="w2")
        px_sb = cst[:, 0:NPX]
        cw_sb = cst[:, NPX:NPX + NCW]
        pe_sb = cst[0:KEEP, NPX + NCW:NPX + NCW + NPE]
        ri_sb = cst[0:KEEP, NPX + NCW + NPE:NPX + NCW + NPE + NRI]
        nc.sync.dma_start(px_sb, pxT)
        nc.sync.dma_start(cw_sb, convw)
        nc.sync.dma_start(pe_sb, peL)
        nc.sync.dma_start(ri_sb, repI)

        x = apool.tile([128, C * T], F32, tag="x", name="x")
        qs = apool.tile([128, C * T], BF16, tag="qs", name="qs")
        ks = apool.tile([128, C * T], BF16, tag="ks", name="ks")
        aO = apool.tile([128, C * T], BF16, tag="aO", name="aO")
        g = apool.tile([128, FC * T], BF16, tag="g", name="g")

        def ln_stats_chunk(st, c):
            ps_sx, ps_sxx = st
            xbc = tpool.tile([128, T], BF16, tag="xb", name="xb")
            nc.vector.tensor_copy(xbc[:], x[:, ts(c, T)])
            x2c = tpool.tile([128, T], BF16, tag="x2", name="x2")
            nc.vector.tensor_mul(x2c[:], xbc[:], xbc[:])
            nc.tensor.matmul(ps_sx[:], ones128[:], xbc[:],
                             start=(c == 0), stop=(c == C - 1))
            nc.tensor.matmul(ps_sxx[:], ones128[:], x2c[:],
                             start=(c == 0), stop=(c == C - 1))

        def layernorm(ln_r, ln_b, h):
            st = (psum((1, T)), psum((1, T)))
            for c in range(C):
                ln_stats_chunk(st, c)
            ps_sx, ps_sxx = st
            m_sb = r2pool.tile([1, T], F32, tag="m_sb", name="m_sb")
            nc.scalar.mul(m_sb[:], ps_sx[:], 1.0 / DIM)
            msq = r1pool.tile([1, T], F32, tag="msq", name="msq")
            nc.vector.tensor_mul(msq[:], m_sb[:], m_sb[:])
            var = r1pool.tile([1, T], F32, tag="var", name="var")
            nc.vector.scalar_tensor_tensor(var[:], ps_sxx[:], 1.0 / DIM, msq[:],
                                           ALU.mult, ALU.subtract)
            sd = r1pool.tile([1, T], F32, tag="sd", name="sd")
            nc.scalar.activation(sd[:], var[:], AFT.Sqrt, bias=eps1[:])
            rstd_b = r2pool.tile([1, T], BF16, tag="rstd_b", name="rstd_b")
            with nc.allow_low_precision(reason="bf16 LN broadcast"):
                nc.vector.reciprocal(rstd_b[:], sd[:])
            mc_b = r2pool.tile([1, T], BF16, tag="mc_b", name="mc_b")
            nc.vector.tensor_mul(mc_b[:], m_sb[:], rstd_b[:])
            # h = x * (s x rstd) - (s x (m*rstd) + (-b) x 1)
            for c in range(C):
                ps_rs = psum()
                nc.tensor.matmul(ps_rs[:], ln_r[:, ts(c, 128)], rstd_b[:],
                                 start=True, stop=True)
                ps_mc = psum()
                nc.tensor.matmul(ps_mc[:], ln_r[:, ts(c, 128)], mc_b[:],
                                 start=True, stop=False)
                nc.tensor.matmul(ps_mc[:], ln_b[:, ts(c, 128)], onesrT[:],
                                 start=False, stop=True)
                tmp = tpool.tile([128, T], F32, tag="tmp", name="tmp")
                nc.vector.tensor_mul(tmp[:], x[:, ts(c, T)], ps_rs[:])
                nc.vector.tensor_sub(h[:, ts(c, T)], tmp[:], ps_mc[:])


        # ---- patch embed: x = convW^T @ patches + pe
        for m in range(C):
            ps = psum()
            for c in range(C):
                nc.tensor.matmul(ps[:], cw_sb[:, c * DIM + m * 128:c * DIM + m * 128 + 128],
                                 px_sb[:, ts(c, T)], start=(c == 0), stop=False)
            nc.tensor.matmul(ps[:], pe_sb[:, ts(m, 128)], ri_sb[:],
                             start=False, stop=True)
            nc.vector.tensor_copy(x[:, ts(m, T)], ps[:])

        for L in range(DEPTH):
            ln1r = lp_pool.tile([1, DIM], BF16, tag="ln1r", name="ln1r")
            nc.sync.dma_start(ln1r[:], ln1r_d[L])
            ln1b = lp_pool.tile([1, DIM], BF16, tag="ln1b", name="ln1b")
            nc.sync.dma_start(ln1b[:], ln1b_d[L])
            ln2r = lp_pool.tile([1, DIM], BF16, tag="ln2r", name="ln2r")
            nc.sync.dma_start(ln2r[:], ln2r_d[L])
            ln2b = lp_pool.tile([1, DIM], BF16, tag="ln2b", name="ln2b")
            nc.sync.dma_start(ln2b[:], ln2b_d[L])
            b1c = lp_pool.tile([128, FC], F32, tag="b1c", name="b1c")
            nc.sync.dma_start(b1c[:], b1_d[L])
            b2p = lp_pool.tile([1, DIM], BF16, tag="b2p", name="b2p")
            nc.sync.dma_start(b2p[:], b2_d[L])

            # ---- LN1
            h = hpool.tile([128, C * T], BF16, tag="h", name="h")
            layernorm(ln1r, ln1b, h)

            # ---- q, k (feature-major [768, 392])
            qw = wq_pool.tile([128, C * DIM], BF16, tag="wqkv", name="wqkv")
            nc.sync.dma_start(qw[:], qw_d[L])
            for m in range(C):
                ps = psum()
                for c in range(C):
                    nc.tensor.matmul(ps[:], qw[:, c * DIM + m * 128:c * DIM + m * 128 + 128],
                                     h[:, ts(c, T)], start=(c == 0), stop=(c == C - 1))
                nc.any.tensor_copy(qs[:, ts(m, T)], ps[:])
            kw = wq_pool.tile([128, C * DIM], BF16, tag="wqkv", name="wqkv")
            nc.sync.dma_start(kw[:], kw_d[L])
            for m in range(C):
                ps = psum()
                for c in range(C):
                    nc.tensor.matmul(ps[:], kw[:, c * DIM + m * 128:c * DIM + m * 128 + 128],
                                     h[:, ts(c, T)], start=(c == 0), stop=(c == C - 1))
                nc.any.tensor_copy(ks[:, ts(m, T)], ps[:])

            vw = wq_pool.tile([128, C * DIM], BF16, tag="wqkv", name="wqkv")
            nc.sync.dma_start(vw[:], vw_d[L])

            # ---- attention, 4-stage software pipeline over images
            vT = [None] * BL
            Etl = [None] * BL
            izl = [None] * BL
            zbl = [None] * BL

            def stage_a(b):       # vT, S^T, exp
                vT[b] = vpool.tile([KEEP, DIM], BF16, tag="vT", name="vT")
                for half in range(2):
                    psv = psum((KEEP, DIM // 2))
                    for c in range(C):
                        nc.tensor.matmul(
                            psv[:],
                            h[:, c * T + b * KEEP:c * T + b * KEEP + KEEP],
                            vw[:, c * DIM + half * 384:c * DIM + half * 384 + 384],
                            start=(c == 0), stop=(c == C - 1))
                    nc.any.tensor_copy(vT[b][:, ts(half, 384)], psv[:])
                pss = [psum((KEEP, 6 * KEEP)), psum((KEEP, 6 * KEEP))]
                for hh in range(NH):
                    j, par = hh // 2, hh % 2
                    nc.tensor.matmul(
                        pss[par][:, ts(j, KEEP)],
                        ks[64 * par:64 * par + 64, j * T + b * KEEP:j * T + b * KEEP + KEEP],
                        qs[64 * par:64 * par + 64, j * T + b * KEEP:j * T + b * KEEP + KEEP],
                        start=True, stop=True)
                E0 = epool.tile([KEEP, 6 * KEEP], BF16, tag="E", name="E")
                E1 = epool.tile([KEEP, 6 * KEEP], BF16, tag="E", name="E")
                nc.scalar.activation(E0[:], pss[0][:], AFT.Exp, bias=zer49[:], scale=0.125)
                nc.scalar.activation(E1[:], pss[1][:], AFT.Exp, bias=zer49[:], scale=0.125)
                Etl[b] = (E0, E1)

            def stage_b(b):       # Z and 1/Z (GpSimd partition reduce; PE/PSUM-free)
                iz0 = r2pool.tile([1, 6 * KEEP], BF16, tag="iz0", name="iz0")
                iz1 = r2pool.tile([1, 6 * KEEP], BF16, tag="iz1", name="iz1")
                izl[b] = (iz0, iz1)
                zbl[b] = (psum((KEEP, 6 * KEEP)), psum((KEEP, 6 * KEEP)))
                for par in range(2):
                    psu = zbl[b][par]
                    nc.tensor.matmul(psu[0:1, :], ones49[:], Etl[b][par][:],
                                     start=True, stop=True)
                    with nc.allow_low_precision(reason="bf16 softmax denominators"):
                        nc.vector.reciprocal(izl[b][par][:], psu[0:1, :])

            def stage_c(b):       # normalize E in place (reuses the Z tile)
                for par in range(2):
                    psb = zbl[b][par]
                    nc.tensor.matmul(psb[:], onesr1[:, :KEEP], izl[b][par][:],
                                     start=True, stop=True)
                    E = Etl[b][par]
                    nc.vector.tensor_mul(E[:], E[:], psb[:])

            def stage_d(b):       # O = V @ A^T, packed into feature-major aO
                pso = psum((128, 6 * KEEP))
                for j in range(C):
                    for par in range(2):
                        hh = 2 * j + par
                        nc.tensor.matmul(
                            pso[64 * par:64 * par + 64, ts(j, KEEP)],
                            vT[b][:, ts(hh, 64)],
                            Etl[b][par][:, ts(j, KEEP)],
                            start=True, stop=True)
                for j in range(C):
                    nc.any.tensor_copy(
                        aO[:, j * T + b * KEEP:j * T + b * KEEP + KEEP],
                        pso[:, ts(j, KEEP)])

            for i in range(BL + 3):
                if i < BL:
                    stage_a(i)
                if 0 <= i - 1 < BL:
                    stage_b(i - 1)
                if 0 <= i - 2 < BL:
                    stage_c(i - 2)
                if 0 <= i - 3 < BL:
                    stage_d(i - 3)

            # ---- proj + residual
            pw = wp_pool.tile([128, C * DIM], BF16, tag="wp", name="wp")
            nc.sync.dma_start(pw[:], pw_d[L])
            for m in range(C):
                ps = psum()
                for c in range(C):
                    nc.tensor.matmul(ps[:], pw[:, c * DIM + m * 128:c * DIM + m * 128 + 128],
                                     aO[:, ts(c, T)], start=(c == 0), stop=(c == C - 1))
                nc.vector.tensor_add(x[:, ts(m, T)], x[:, ts(m, T)], ps[:])

            # ---- LN2
            h2 = hpool.tile([128, C * T], BF16, tag="h", name="h")
            layernorm(ln2r, ln2b, h2)

            # ---- FFN phase 1: g = gelu(h2 @ w1 + b1), 24 chunks
            for c in range(FC):
                if c % 6 == 0:
                    w1t = w1_pool.tile([128, C * DIM], BF16, tag="w1", name="w1")
                    nc.sync.dma_start(w1t[:], w1_d[L, c // 6])
                psf = psum()
                for ci in range(C):
                    nc.tensor.matmul(
                        psf[:], w1t[:, ci * DIM + (c % 6) * 128:ci * DIM + (c % 6) * 128 + 128],
                        h2[:, ts(ci, T)], start=(ci == 0), stop=(ci == C - 1))
                nc.scalar.activation(g[:, ts(c, T)], psf[:], AFT.Gelu_apprx_tanh,
                                     bias=b1c[:, c:c + 1])

            # ---- FFN phase 2: x += g @ w2 + b2
            w2t = w2_pool.tile([128, 4 * C * DIM], BF16, tag="w2", name="w2")
            nc.sync.dma_start(w2t[:], w2_d[L])
            for m in range(C):
                ps = psum()
                nc.tensor.matmul(ps[:], b2p[:, ts(m, 128)], onesrT[:],
                                 start=True, stop=False)
                for c in range(FC):
                    nc.tensor.matmul(
                        ps[:], w2t[:, c * DIM + m * 128:c * DIM + m * 128 + 128],
                        g[:, ts(c, T)], start=False, stop=(c == FC - 1))
                nc.vector.tensor_add(x[:, ts(m, T)], x[:, ts(m, T)], ps[:])

        for c in range(C):
            nc.sync.dma_start(out_d[c * 128:(c + 1) * 128, :], x[:, ts(c, T)])
        es.close()

    nc.compile()
    return nc


def _prep_shared(inputs):
    """Host-side packing of weights (shared across cores), bf16."""
    sh = {}
    sh["convw"] = _chunk_pack(
        np.asarray(inputs["conv_w"], np.float32).reshape(P * P * 3, DIM), DIM).astype(bf16)
    qkv = np.asarray(inputs["qkv_w"], np.float32)
    sh["qw"] = np.stack([_chunk_pack(qkv[L, :, :DIM], DIM) for L in range(DEPTH)]).astype(bf16)
    sh["kw"] = np.stack([_chunk_pack(qkv[L, :, DIM:2 * DIM], DIM) for L in range(DEPTH)]).astype(bf16)
    sh["vw"] = np.stack([_chunk_pack(qkv[L, :, 2 * DIM:], DIM) for L in range(DEPTH)]).astype(bf16)
    pwt = np.asarray(inputs["proj_w"], np.float32)
    sh["pw"] = np.stack([_chunk_pack(pwt[L], DIM) for L in range(DEPTH)]).astype(bf16)
    w1 = np.asarray(inputs["w1"], np.float32)
    sh["w1q"] = np.stack([
        np.stack([_chunk_pack(w1[L][:, 768 * qt:768 * (qt + 1)], DIM) for qt in range(4)])
        for L in range(DEPTH)]).astype(bf16)
    w2 = np.asarray(inputs["w2"], np.float32)
    sh["w2q"] = np.stack([
        np.concatenate([_chunk_pack(w2[L][768 * qt:768 * (qt + 1), :], DIM)
                        for qt in range(4)], axis=1)
        for L in range(DEPTH)]).astype(bf16)
    sh["b2p"] = np.asarray(inputs["b2"], np.float32).reshape(DEPTH, 1, DIM).astype(bf16)
    sh["b1c"] = np.ascontiguousarray(
        np.asarray(inputs["b1"], np.float32).reshape(DEPTH, FC, 128).transpose(0, 2, 1))
    sh["ln1r"] = np.asarray(inputs["ln1_s"], np.float32).reshape(DEPTH, 1, DIM).astype(bf16)
    sh["ln1b"] = (-np.asarray(inputs["ln1_b"], np.float32)).reshape(DEPTH, 1, DIM).astype(bf16)
    sh["ln2r"] = np.asarray(inputs["ln2_s"], np.float32).reshape(DEPTH, 1, DIM).astype(bf16)
    sh["ln2b"] = (-np.asarray(inputs["ln2_b"], np.float32)).reshape(DEPTH, 1, DIM).astype(bf16)
    return sh


def kernel(**inputs) -> np.ndarray:
    if "nc" not in _cached:
        _cached["nc"] = _build()
    nc = _cached["nc"]

    mask = np.minimum(np.asarray(inputs["mask_idx"]).astype(np.int64), 196)
    mask = np.maximum(mask, 1)
    pidx = mask - 1
    imgs = np.asarray(inputs["inputs"], np.float32)
    pat = imgs.reshape(B, 14, P, 14, P, 3).transpose(0, 1, 3, 2, 4, 5).reshape(B, 196, P * P * 3)
    pat = pat[:, pidx]                                     # [B, 49, 768]
    pe = np.asarray(inputs["pe"], np.float32)[0, mask]     # [49, 768]

    sh = _prep_shared(inputs)
    sh["peL"] = pe.astype(bf16)
    sh["repI"] = np.tile(np.eye(KEEP, dtype=np.float32), (1, BL)).astype(bf16)

    in_maps = []
    for r in range(NCORES):
        pxT = pat[r * BL:(r + 1) * BL].reshape(T, P * P * 3).T   # [768, 392]
        m = dict(sh)
        m["pxT"] = _chunk_pack(np.ascontiguousarray(pxT), T).astype(bf16)
        in_maps.append(m)

    res = run_bass_kernel_spmd(nc, in_maps, core_ids=list(range(NCORES)),
                               trace=_cached.get("trace", False),
                               tmpdir=_cached.get("tmpdir"))
    _cached["last_result"] = res

    out = np.empty((B, KEEP, DIM), np.float32)
    for r in range(NCORES):
        o = res.results[r]["out"]                          # [768, 392]
        out[r * BL:(r + 1) * BL] = o.T.reshape(BL, KEEP, DIM)
    return out



# revision 4
# speedup vs baseline: 1.1262x; 1.1262x over previous
"""MAE ViT-Base encoder (masked, KEEP=49) on 8 TRN2 NeuronCores.

Data-parallel over batch (8 images/core). Feature-major activations
[768, 392] on-chip; bf16 matmul inputs, fp32 PSUM accumulation, fp32
residual stream. LN scales are folded into the following weight matrices
on host; LN statistics are computed with a replicated ones[128,128]
matmul so the whole normalization chain is elementwise [128,T] work
(no single-partition reciprocals, no rank-1 broadcast matmuls).
rstd = exp(-0.5*ln(var+eps)) keeps every scalar-engine op in the
natural_log_exp table set (shared with softmax exp); gelu is the only
other set and both switches are preloaded by dummy ops off the critical
path. Softmax normalization is deferred past O = V@E^ (unnormalized)
and applied as one fused column-scale into an image-major aO buffer.
"""

import numpy as np
import ml_dtypes

import concourse.bass as bass
import concourse.mybir as mybir
import concourse.tile as tile
from concourse import bacc
from concourse.bass import ts
from concourse.bass_utils import run_bass_kernel_spmd

AFT = mybir.ActivationFunctionType
BF16 = mybir.dt.bfloat16
F32 = mybir.dt.float32
ALU = mybir.AluOpType

B, P, DIM, DEPTH, NH, DH, FF = 64, 16, 768, 12, 12, 64, 3072
KEEP = 49
NCORES = 8
BL = B // NCORES          # 8 images per core
T = BL * KEEP             # 392 tokens per core
C = DIM // 128            # 6 feature chunks
FC = FF // 128            # 24 ffn chunks
IM = C * KEEP             # 294 columns per image (6 heads x 49 q)
EPS = 1e-9

bf16 = ml_dtypes.bfloat16
_cached = {}


def _chunk_pack(w, cols):
    """[768, cols] -> [128, 6*cols] with tile[p, c*cols+x] = w[c*128+p, x]."""
    return np.ascontiguousarray(
        w.reshape(C, 128, cols).transpose(1, 0, 2).reshape(128, C * cols))


def _build(qkv_bias: bool, b2_bias: bool):
    nc = bacc.Bacc("TRN2", target_bir_lowering=False, debug=False,
                   enable_asserts=False, num_devices=NCORES)

    def din(name, shape, dt=BF16):
        return nc.dram_tensor(name, shape, dt, kind="ExternalInput").ap()

    pxT = din("pxT", [128, C * T])
    convw = din("convw", [128, C * DIM])
    peL = din("peL", [KEEP, DIM])
    repI = din("repI", [KEEP, T])
    qw_d = din("qw", [DEPTH, 128, C * DIM])
    kw_d = din("kw", [DEPTH, 128, C * DIM])
    vw_d = din("vw", [DEPTH, 128, C * DIM])
    pw_d = din("pw", [DEPTH, 128, C * DIM])
    w1_d = din("w1q", [DEPTH, 4, 128, C * DIM])
    w2_d = din("w2q", [DEPTH, 4, 128, C * DIM])
    b1_d = din("b1c", [DEPTH, 128, FC], F32)
    if qkv_bias:
        qb_d = din("qb", [DEPTH, 1, DIM])
        kb_d = din("kb", [DEPTH, 1, DIM])
        vb_d = din("vb", [DEPTH, 1, DIM])
    if b2_bias:
        b2_d = din("b2p", [DEPTH, 1, DIM])
    out_d = nc.dram_tensor("out", [DIM, T], F32, kind="ExternalOutput").ap()

    with tile.TileContext(nc) as tc:
        from contextlib import ExitStack
        es = ExitStack()
        cpool = es.enter_context(tc.tile_pool(name="consts", bufs=1))
        cstp = es.enter_context(tc.tile_pool(name="pconsts", bufs=1))
        apool = es.enter_context(tc.tile_pool(name="acts", bufs=1))
        hpool = es.enter_context(tc.tile_pool(name="h", bufs=1))
        lnpool = es.enter_context(tc.tile_pool(name="ln", bufs=2))
        statp = es.enter_context(tc.tile_pool(name="stat", bufs=3))
        tpool = es.enter_context(tc.tile_pool(name="tmp", bufs=3))
        vpool = es.enter_context(tc.tile_pool(name="vt", bufs=5))
        epool = es.enter_context(tc.tile_pool(name="etiles", bufs=8))
        izpool = es.enter_context(tc.tile_pool(name="iz", bufs=6))
        wq_pool = es.enter_context(tc.tile_pool(name="wqkv", bufs=3))
        w1_pool = es.enter_context(tc.tile_pool(name="w1", bufs=2))
        w2_pool = es.enter_context(tc.tile_pool(name="w2", bufs=2))
        lp_pool = es.enter_context(tc.tile_pool(name="lparam", bufs=2))
        pspool = es.enter_context(tc.tile_pool(name="ps", bufs=8, space="PSUM"))

        def psum(shape=(128, T)):
            return pspool.tile(list(shape), F32, tag="ps", name="ps")

        # ---- small constants
        onessq = cpool.tile([128, 128], BF16, tag="onessq", name="onessq")
        nc.vector.memset(onessq[:], 1.0)
        ones49 = cpool.tile([KEEP, 1], BF16, tag="ones49", name="ones49")
        nc.vector.memset(ones49[:], 1.0)
        ones64r = cpool.tile([1, 64], BF16, tag="ones64r", name="ones64r")
        nc.vector.memset(ones64r[:], 1.0)
        onesrT = cpool.tile([1, T], BF16, tag="onesrT", name="onesrT")
        nc.vector.memset(onesrT[:], 1.0)
        epsc = cpool.tile([128, 1], F32, tag="epsc", name="epsc")
        nc.vector.memset(epsc[:], EPS)
        dum = cpool.tile([1, 2], F32, tag="dum", name="dum")
        nc.vector.memset(dum[:], 1.0)

        def preload(func):
            # dummy activation to pull the table set load off the critical path
            nc.scalar.activation(dum[:, 1:2], dum[:, 0:1], func)

        # ---- patch-embed constants (own pool; used once)
        NPX, NCW, NPE, NRI = C * T, C * DIM, DIM, T
        cst = cstp.tile([128, NPX + NCW + NPE + NRI], BF16, tag="cst", name="cst")
        px_sb = cst[:, 0:NPX]
        cw_sb = cst[:, NPX:NPX + NCW]
        pe_sb = cst[0:KEEP, NPX + NCW:NPX + NCW + NPE]
        ri_sb = cst[0:KEEP, NPX + NCW + NPE:NPX + NCW + NPE + NRI]
        nc.sync.dma_start(px_sb, pxT)
        nc.sync.dma_start(cw_sb, convw)
        nc.sync.dma_start(pe_sb, peL)
        nc.sync.dma_start(ri_sb, repI)

        x = apool.tile([128, C * T], F32, tag="x", name="x")
        qs = apool.tile([128, C * T], BF16, tag="qs", name="qs")
        ks = apool.tile([128, C * T], BF16, tag="ks", name="ks")
        aO = apool.tile([128, BL * IM], BF16, tag="aO", name="aO")
        g = apool.tile([128, FC * T], BF16, tag="g", name="g")

        def chunk_stats(st, m):
            """Accumulate sum/sumsq of x chunk m into replicated [128,T] psums."""
            ps_sx, ps_sxx = st
            xb = statp.tile([128, T], BF16, tag="xb", name="xb")
            nc.scalar.copy(xb[:], x[:, ts(m, T)])
            x2 = statp.tile([128, T], BF16, tag="x2", name="x2")
            nc.scalar.square(x2[:], xb[:])
            nc.tensor.matmul(ps_sx[:], onessq[:], xb[:],
                             start=(m == 0), stop=(m == C - 1))
            nc.tensor.matmul(ps_sxx[:], onessq[:], x2[:],
                             start=(m == 0), stop=(m == C - 1))

        def ln_chain(st, h):
            """st -> h = (x - m) * rsqrt(var+eps), all elementwise [128,T]."""
            ps_sx, ps_sxx = st
            msq = lnpool.tile([128, T], F32, tag="msq", name="msq")
            nc.scalar.activation(msq[:], ps_sx[:], AFT.Square, scale=1.0 / DIM)
            var = lnpool.tile([128, T], F32, tag="var", name="var")
            nc.vector.scalar_tensor_tensor(var[:], ps_sxx[:], 1.0 / DIM, msq[:],
                                           ALU.mult, ALU.subtract)
            lnv = lnpool.tile([128, T], F32, tag="lnv", name="lnv")
            nc.scalar.activation(lnv[:], var[:], AFT.Ln, bias=epsc[:])
            rstd = lnpool.tile([128, T], BF16, tag="rstd", name="rstd")
            nc.scalar.activation(rstd[:], lnv[:], AFT.Exp, scale=-0.5)
            mrstd = lnpool.tile([128, T], F32, tag="mrstd", name="mrstd")
            nc.vector.scalar_tensor_tensor(mrstd[:], ps_sx[:], 1.0 / DIM, rstd[:],
                                           ALU.mult, ALU.mult)
            for c in range(C):
                tmp = tpool.tile([128, T], F32, tag="lt", name="lt")
                nc.vector.tensor_mul(tmp[:], x[:, ts(c, T)], rstd[:])
                nc.vector.tensor_sub(h[:, ts(c, T)], tmp[:], mrstd[:])

        def dense_couter(wt, h, dst, bias_sb):
            """dst[128, C*T] = wt^T @ h (+ bias row), contraction-outer order."""
            pss = [psum() for _ in range(C)]
            for c in range(C):
                for m in range(C):
                    nc.tensor.matmul(
                        pss[m][:], wt[:, c * DIM + m * 128:c * DIM + m * 128 + 128],
                        h[:, ts(c, T)], start=(c == 0),
                        stop=(c == C - 1 and bias_sb is None))
            if bias_sb is not None:
                for m in range(C):
                    nc.tensor.matmul(pss[m][:], bias_sb[:, ts(m, 128)], onesrT[:],
                                     start=False, stop=True)
            for m in range(C):
                nc.vector.tensor_copy(dst[:, ts(m, T)], pss[m][:])

        # ---- patch embed: x = convW^T @ patches + pe; feed LN1 stats of layer 0
        preload(AFT.Ln)
        st1 = (psum(), psum())
        for m in range(C):
            ps = psum()
            for c in range(C):
                nc.tensor.matmul(ps[:], cw_sb[:, c * DIM + m * 128:c * DIM + m * 128 + 128],
                                 px_sb[:, ts(c, T)], start=(c == 0), stop=False)
            nc.tensor.matmul(ps[:], pe_sb[:, ts(m, 128)], ri_sb[:],
                             start=False, stop=True)
            nc.vector.tensor_copy(x[:, ts(m, T)], ps[:])
            chunk_stats(st1, m)

        # layer-0 weight prefetch
        qwt = wq_pool.tile([128, C * DIM], BF16, tag="wqkv", name="wqkv")
        nc.sync.dma_start(qwt[:], qw_d[0])
        kwt = wq_pool.tile([128, C * DIM], BF16, tag="wqkv", name="wqkv")
        nc.sync.dma_start(kwt[:], kw_d[0])
        vwt = wq_pool.tile([128, C * DIM], BF16, tag="wqkv", name="wqkv")
        nc.sync.dma_start(vwt[:], vw_d[0])

        for L in range(DEPTH):
            b1c = lp_pool.tile([128, FC], F32, tag="b1c", name="b1c")
            nc.sync.dma_start(b1c[:], b1_d[L])
            if qkv_bias:
                qb = lp_pool.tile([1, DIM], BF16, tag="qb", name="qb")
                nc.sync.dma_start(qb[:], qb_d[L])
                kb = lp_pool.tile([1, DIM], BF16, tag="kb", name="kb")
                nc.sync.dma_start(kb[:], kb_d[L])
                vb = lp_pool.tile([1, DIM], BF16, tag="vb", name="vb")
                nc.sync.dma_start(vb[:], vb_d[L])
            if b2_bias:
                b2p = lp_pool.tile([1, DIM], BF16, tag="b2p", name="b2p")
                nc.sync.dma_start(b2p[:], b2_d[L])

            # ---- LN1 + q/k projections (consume h chunk-by-chunk)
            h = hpool.tile([128, C * T], BF16, tag="h", name="h")
            ln_chain(st1, h)
            dense_couter(qwt, h, qs, qb if qkv_bias else None)
            dense_couter(kwt, h, ks, kb if qkv_bias else None)

            # prefetch proj weights (slot freed after layer L-1's proj)
            pwt = wq_pool.tile([128, C * DIM], BF16, tag="wqkv", name="wqkv")
            nc.sync.dma_start(pwt[:], pw_d[L])

            # ---- attention pipeline over images
            vT = [None] * BL
            Etl = [None] * BL
            izl = [None] * BL
            psol = [None] * BL

            def stage_a(b):       # vT, S^T, exp
                vT[b] = vpool.tile([KEEP, DIM], BF16, tag="vT", name="vT")
                for half in range(2):
                    psv = psum((KEEP, DIM // 2))
                    for c in range(C):
                        nc.tensor.matmul(
                            psv[:],
                            h[:, c * T + b * KEEP:c * T + b * KEEP + KEEP],
                            vwt[:, c * DIM + half * 384:c * DIM + half * 384 + 384],
                            start=(c == 0), stop=(c == C - 1))
                    if qkv_bias:
                        nc.tensor.matmul(
                            psv[:], ones64r[:, :KEEP],
                            vb[:, half * 384:half * 384 + 384],
                            start=False, stop=True)
                    nc.scalar.copy(vT[b][:, ts(half, 384)], psv[:])
                pss = [psum((KEEP, IM)), psum((KEEP, IM))]
                for hh in range(NH):
                    j, par = hh // 2, hh % 2
                    nc.tensor.matmul(
                        pss[par][:, ts(j, KEEP)],
                        ks[64 * par:64 * par + 64, j * T + b * KEEP:j * T + b * KEEP + KEEP],
                        qs[64 * par:64 * par + 64, j * T + b * KEEP:j * T + b * KEEP + KEEP],
                        start=True, stop=True)
                E0 = epool.tile([KEEP, IM], BF16, tag="E", name="E")
                E1 = epool.tile([KEEP, IM], BF16, tag="E", name="E")
                nc.scalar.activation(E0[:], pss[0][:], AFT.Exp, scale=0.125)
                nc.scalar.activation(E1[:], pss[1][:], AFT.Exp, scale=0.125)
                Etl[b] = (E0, E1)

            def stage_b1(b):      # Z rows + fast reciprocal (fp32) + bf16 cast
                izl[b] = []
                for par in range(2):
                    zps = psum((1, IM))
                    nc.tensor.matmul(zps[:], ones49[:], Etl[b][par][:],
                                     start=True, stop=True)
                    iz32 = izpool.tile([1, IM], F32, tag="iz32", name="iz32")
                    nc.vector.reciprocal_approx_fast(iz32[:], zps[:])
                    iz16 = izpool.tile([1, IM], BF16, tag="iz16", name="iz16")
                    nc.vector.tensor_copy(iz16[:], iz32[:])
                    izl[b].append(iz16)

            def stage_b2(b):      # broadcast 1/Z across the 128 feature rows
                izb = psum((128, IM))
                nc.tensor.matmul(izb[0:64, :], ones64r[:], izl[b][0][:],
                                 start=True, stop=True, skip_group_check=True)
                nc.tensor.matmul(izb[64:128, :], ones64r[:], izl[b][1][:],
                                 start=True, stop=True, skip_group_check=True)
                izb_sb = izpool.tile([128, IM], BF16, tag="izb", name="izb")
                nc.scalar.copy(izb_sb[:], izb[:])
                izl[b] = izb_sb

            def stage_c(b):       # O~ = V @ E^T, then fused normalize into aO
                pso = psum((128, IM))
                for j in range(C):
                    for par in range(2):
                        hh = 2 * j + par
                        nc.tensor.matmul(
                            pso[64 * par:64 * par + 64, ts(j, KEEP)],
                            vT[b][:, ts(hh, 64)],
                            Etl[b][par][:, ts(j, KEEP)],
                            start=True, stop=True)
                psol[b] = pso
                nc.vector.tensor_mul(aO[:, b * IM:(b + 1) * IM], pso[:], izl[b][:])

            for i in range(BL + 2):
                if 0 <= i - 1 < BL:
                    stage_b1(i - 1)
                if i < BL:
                    stage_a(i)
                if 0 <= i - 1 < BL:
                    stage_b2(i - 1)
                if 0 <= i - 2 < BL:
                    stage_c(i - 2)

            # next-layer qkv weight prefetch (slots rotate through the pool)
            if L + 1 < DEPTH:
                qwt_n = wq_pool.tile([128, C * DIM], BF16, tag="wqkv", name="wqkv")
                nc.sync.dma_start(qwt_n[:], qw_d[L + 1])
                kwt_n = wq_pool.tile([128, C * DIM], BF16, tag="wqkv", name="wqkv")
                nc.sync.dma_start(kwt_n[:], kw_d[L + 1])

            # ---- proj + residual, feeding LN2 stats
            aOv = aO[:].rearrange("p (b c k) -> p b c k", b=BL, c=C, k=KEEP)
            st2 = (psum(), psum())
            for m in range(C):
                ps = psum()
                for c in range(C):
                    nc.tensor.matmul(ps[:], pwt[:, c * DIM + m * 128:c * DIM + m * 128 + 128],
                                     aOv[:, :, c, :], start=(c == 0), stop=(c == C - 1))
                nc.vector.tensor_add(x[:, ts(m, T)], x[:, ts(m, T)], ps[:])
                chunk_stats(st2, m)

            # ---- LN2
            h2 = hpool.tile([128, C * T], BF16, tag="h", name="h")
            ln_chain(st2, h2)
            preload(AFT.Gelu_apprx_tanh)

            # ---- FFN1: g = gelu(h2 @ w1 + b1), 4 quarter-groups, c-outer
            w1t = w1_pool.tile([128, C * DIM], BF16, tag="w1", name="w1")
            nc.sync.dma_start(w1t[:], w1_d[L, 0])
            w2t0 = w2_pool.tile([128, C * DIM], BF16, tag="w2", name="w2")
            nc.sync.dma_start(w2t0[:], w2_d[L, 0])
            for grp in range(4):
                if grp + 1 < 4:
                    w1t_n = w1_pool.tile([128, C * DIM], BF16, tag="w1", name="w1")
                    nc.sync.dma_start(w1t_n[:], w1_d[L, grp + 1])
                psf = [psum() for _ in range(C)]
                for c in range(C):
                    for m in range(C):
                        nc.tensor.matmul(
                            psf[m][:], w1t[:, c * DIM + m * 128:c * DIM + m * 128 + 128],
                            h2[:, ts(c, T)], start=(c == 0), stop=(c == C - 1))
                for m in range(C):
                    fc = grp * C + m
                    nc.scalar.activation(g[:, ts(fc, T)], psf[m][:],
                                         AFT.Gelu_apprx_tanh, bias=b1c[:, fc:fc + 1])
                if grp + 1 < 4:
                    w1t = w1t_n
            preload(AFT.Ln)

            # ---- FFN2: x += g @ w2 (+ b2), c-outer with quartered weights
            last = (L == DEPTH - 1)
            psm = [psum() for _ in range(C)]
            if b2_bias:
                for m in range(C):
                    nc.tensor.matmul(psm[m][:], b2p[:, ts(m, 128)], onesrT[:],
                                     start=True, stop=False)
            w2t = w2t0
            for qt in range(4):
                if qt + 1 < 4:
                    w2t_n = w2_pool.tile([128, C * DIM], BF16, tag="w2", name="w2")
                    nc.sync.dma_start(w2t_n[:], w2_d[L, qt + 1])
                for cl in range(C):
                    c = qt * C + cl
                    for m in range(C):
                        nc.tensor.matmul(
                            psm[m][:], w2t[:, cl * DIM + m * 128:cl * DIM + m * 128 + 128],
                            g[:, ts(c, T)], start=(c == 0 and not b2_bias),
                            stop=(c == FC - 1))
                if qt + 1 < 4:
                    w2t = w2t_n
            if not last:
                st1 = (psum(), psum())
            for m in range(C):
                nc.vector.tensor_add(x[:, ts(m, T)], x[:, ts(m, T)], psm[m][:])
                if not last:
                    chunk_stats(st1, m)
                else:
                    nc.sync.dma_start(out_d[m * 128:(m + 1) * 128, :], x[:, ts(m, T)])
            if not last:
                qwt, kwt = qwt_n, kwt_n
                vwt = wq_pool.tile([128, C * DIM], BF16, tag="wqkv", name="wqkv")
                nc.sync.dma_start(vwt[:], vw_d[L + 1])
        es.close()

    nc.compile()
    return nc


def _prep_shared(inputs):
    """Host-side packing of weights (shared across cores), bf16.

    LN scales are folded into the following matmul weights; LN biases fold
    into b1 (always) and into explicit qkv bias rows (only when nonzero).
    """
    sh = {}
    sh["convw"] = _chunk_pack(
        np.asarray(inputs["conv_w"], np.float32).reshape(P * P * 3, DIM), DIM).astype(bf16)
    ln1s = np.asarray(inputs["ln1_s"], np.float32)
    ln1b = np.asarray(inputs["ln1_b"], np.float32)
    ln2s = np.asarray(inputs["ln2_s"], np.float32)
    ln2b = np.asarray(inputs["ln2_b"], np.float32)
    qkv = np.asarray(inputs["qkv_w"], np.float32)
    qkv_s = qkv * ln1s[:, :, None]
    sh["qw"] = np.stack([_chunk_pack(qkv_s[L, :, :DIM], DIM) for L in range(DEPTH)]).astype(bf16)
    sh["kw"] = np.stack([_chunk_pack(qkv_s[L, :, DIM:2 * DIM], DIM) for L in range(DEPTH)]).astype(bf16)
    sh["vw"] = np.stack([_chunk_pack(qkv_s[L, :, 2 * DIM:], DIM) for L in range(DEPTH)]).astype(bf16)
    pwt = np.asarray(inputs["proj_w"], np.float32)
    sh["pw"] = np.stack([_chunk_pack(pwt[L], DIM) for L in range(DEPTH)]).astype(bf16)
    w1 = np.asarray(inputs["w1"], np.float32)
    w1s = w1 * ln2s[:, :, None]
    sh["w1q"] = np.stack([
        np.stack([_chunk_pack(w1s[L][:, 768 * qt:768 * (qt + 1)], DIM) for qt in range(4)])
        for L in range(DEPTH)]).astype(bf16)
    w2 = np.asarray(inputs["w2"], np.float32)
    sh["w2q"] = np.stack([
        np.stack([_chunk_pack(w2[L][768 * qt:768 * (qt + 1), :], DIM) for qt in range(4)])
        for L in range(DEPTH)]).astype(bf16)
    b1p = np.asarray(inputs["b1"], np.float32) + np.einsum('ld,ldf->lf', ln2b, w1)
    sh["b1c"] = np.ascontiguousarray(
        b1p.reshape(DEPTH, FC, 128).transpose(0, 2, 1))
    flags = {}
    flags["qkv_bias"] = bool(np.abs(ln1b).max() > 0)
    flags["b2_bias"] = bool(np.abs(np.asarray(inputs["b2"])).max() > 0)
    if flags["qkv_bias"]:
        qkvb = np.einsum('ld,ldf->lf', ln1b, qkv)      # [DEPTH, 3*DIM]
        sh["qb"] = qkvb[:, :DIM].reshape(DEPTH, 1, DIM).astype(bf16)
        sh["kb"] = qkvb[:, DIM:2 * DIM].reshape(DEPTH, 1, DIM).astype(bf16)
        sh["vb"] = qkvb[:, 2 * DIM:].reshape(DEPTH, 1, DIM).astype(bf16)
    if flags["b2_bias"]:
        sh["b2p"] = np.asarray(inputs["b2"], np.float32).reshape(DEPTH, 1, DIM).astype(bf16)
    return sh, flags


def kernel(**inputs) -> np.ndarray:
    sh, flags = _prep_shared(inputs)
    key = (flags["qkv_bias"], flags["b2_bias"])
    if _cached.get("key") != key:
        _cached["nc"] = _build(*key)
        _cached["key"] = key
    nc = _cached["nc"]

    mask = np.minimum(np.asarray(inputs["mask_idx"]).astype(np.int64), 196)
    mask = np.maximum(mask, 1)
    pidx = mask - 1
    imgs = np.asarray(inputs["inputs"], np.float32)
    pat = imgs.reshape(B, 14, P, 14, P, 3).transpose(0, 1, 3, 2, 4, 5).reshape(B, 196, P * P * 3)
    pat = pat[:, pidx]                                     # [B, 49, 768]
    pe = np.asarray(inputs["pe"], np.float32)[0, mask]     # [49, 768]

    sh["peL"] = pe.astype(bf16)
    sh["repI"] = np.tile(np.eye(KEEP, dtype=np.float32), (1, BL)).astype(bf16)

    in_maps = []
    for r in range(NCORES):
        pxT = pat[r * BL:(r + 1) * BL].reshape(T, P * P * 3).T   # [768, 392]
        m = dict(sh)
        m["pxT"] = _chunk_pack(np.ascontiguousarray(pxT), T).astype(bf16)
        in_maps.append(m)

    res = run_bass_kernel_spmd(nc, in_maps, core_ids=list(range(NCORES)),
                               trace=_cached.get("trace", False),
                               tmpdir=_cached.get("tmpdir"))
    _cached["last_result"] = res

    out = np.empty((B, KEEP, DIM), np.float32)
    for r in range(NCORES):
        o = res.results[r]["out"]                          # [768, 392]
        out[r * BL:(r + 1) * BL] = o.T.reshape(BL, KEEP, DIM)
    return out
